# revision 1
# baseline (speedup 1.0000x reference)
"""Trainium2 Bass kernel for nn_BasicBlock (WeightNet/CondConv-style block).

Data parallel over batch: 32 samples -> 8 cores x 4 samples.
Per core, per sample:
  gap   = mean(x, HW) @ reduce_w.T + reduce_b                  (PE + DVE)
  a_wn  = sigmoid(gap @ fc1_w.T + fc1_b)                       (PE + ACT)
  W_wn  = einsum('gi,goi->go', a, w2) per-sample 3x3 kernels   (DVE)
  out   = relu(bn1(conv(x, W1)))                               (PE + ACT)
  out   = bn2(conv(out, W2)) + x; relu                         (PE + ACT + DVE)
Conv implemented as 9 shifted bf16 matmuls accumulating in PSUM, padded
58x58 image layout in SBUF. Static weights host-pre-packed (layout only).
"""

import sys

sys.path.insert(0, "/opt/trn_rl_repo")

import numpy as np
import ml_dtypes

import concourse.bass as bass
import concourse.tile as tile
from concourse import bacc, mybir
from concourse import bass_utils

F32 = mybir.dt.float32
BF16 = mybir.dt.bfloat16
AF = mybir.ActivationFunctionType

B, C, H, W = 32, 256, 56, 56
NCORES = 8
BL = B // NCORES          # samples per core
HP, WP = H + 2, W + 2     # padded 58x58
NPIX = H * W              # 3136
NPPAD = HP * WP           # 3364
NT = 7                    # h-tiles of 8 rows
TROWS = 8
NFREE = TROWS * W         # 448 columns per matmul
EPS = 1e-5


def build_program():
    nc = bacc.Bacc("TRN2", target_bir_lowering=False, debug=False,
                   num_devices=NCORES)

    x4 = nc.dram_tensor("x4", [BL, C, H, W], F32, kind="ExternalInput").ap()
    x4b = nc.dram_tensor("x4b", [BL, C, H, W], BF16, kind="ExternalInput").ap()
    out4 = nc.dram_tensor("out4", [BL, C, H, W], F32, kind="ExternalOutput").ap()
    rwT = nc.dram_tensor("rwT", [2, 128, 16], F32, kind="ExternalInput").ap()
    rb = nc.dram_tensor("rb", [16, 1], F32, kind="ExternalInput").ap()
    fc1wT = [nc.dram_tensor(f"fc1wT{n}", [16, 4096], BF16, kind="ExternalInput").ap()
             for n in (1, 2)]
    fc1b = [nc.dram_tensor(f"fc1b{n}", [128, 32], F32, kind="ExternalInput").ap()
            for n in (1, 2)]
    w2p = [nc.dram_tensor(f"w2p{n}", [2, 128, 4 * 9 * 256], BF16,
                          kind="ExternalInput").ap() for n in (1, 2)]
    bns = [nc.dram_tensor(f"bns{n}", [2, 128, 1], F32, kind="ExternalInput").ap()
           for n in (1, 2)]
    bnb = [nc.dram_tensor(f"bnb{n}", [2, 128, 1], F32, kind="ExternalInput").ap()
           for n in (1, 2)]

    with tile.TileContext(nc) as tc:
        build_body(tc, x4, x4b, out4, rwT, rb, fc1wT, fc1b, w2p, bns, bnb)

    nc.compile()
    return nc


def build_body(tc, x4, x4b, out4, rwT, rb, fc1wT, fc1b, w2p, bns, bnb):
    nc = tc.nc
    from contextlib import ExitStack
    ctx = ExitStack()

    cpool = ctx.enter_context(tc.tile_pool(name="consts", bufs=1))
    xpad_p = ctx.enter_context(tc.tile_pool(name="xpad", bufs=4))
    o1pad_p = ctx.enter_context(tc.tile_pool(name="o1pad", bufs=4))
    aexp_p = ctx.enter_context(tc.tile_pool(name="aexp", bufs=2))
    wgen_p = ctx.enter_context(tc.tile_pool(name="wgen", bufs=6))
    wtmp_p = ctx.enter_context(tc.tile_pool(name="wtmp", bufs=2))
    small_p = ctx.enter_context(tc.tile_pool(name="small", bufs=2))
    stage_p = ctx.enter_context(tc.tile_pool(name="stage", bufs=2))
    avlin_p = ctx.enter_context(tc.tile_pool(name="avlinp", bufs=1))
    xstage_p = ctx.enter_context(tc.tile_pool(name="xstage", bufs=1))
    psum_p = ctx.enter_context(tc.tile_pool(name="psum", bufs=5, space="PSUM"))
    psmall_p = ctx.enter_context(tc.tile_pool(name="psmall", bufs=1, space="PSUM"))
    dram_p = ctx.enter_context(tc.tile_pool(name="dscratch", bufs=2, space="DRAM"))

    # sample-0 chunk-0 staging load first: it heads the critical chain
    xs_pre = xstage_p.tile([128, NPIX], BF16, tag="xstage")
    nc.sync.dma_start(xs_pre[:],
                      x4b[0, 0:128].rearrange("c h w -> c (h w)"))

    # ---- resident constants (w2sb DMAs deferred for startup overlap) ----
    w2sb = []   # [wn][chunk][i] -> [128, 2304] bf16 (k*256+co)
    for n in range(2):
        per = []
        for c in range(2):
            blocks = []
            for i in range(4):
                w2t = cpool.tile([128, 2304], BF16, tag=f"w2sb{n}{c}{i}")
                blocks.append(w2t)
            per.append(blocks)
        w2sb.append(per)

    def load_w2sb(n):
        for c in range(2):
            for i in range(4):
                nc.sync.dma_start(w2sb[n][c][i][:],
                                  w2p[n][c][:, 2304 * i:2304 * (i + 1)])
    rwT_sb = []
    for c in range(2):
        t = cpool.tile([128, 16], F32, tag=f"rwT{c}")
        nc.sync.dma_start(t[:], rwT[c])
        rwT_sb.append(t)
    rb_sb = cpool.tile([16, 1], F32, tag="rb")
    nc.sync.dma_start(rb_sb[:], rb)
    fc1wT_sb, fc1b_sb, bns_sb, bnb_sb = [], [], [], []
    for n in range(2):
        t = cpool.tile([16, 4096], BF16, tag=f"fc1wT{n}")
        if n == 0:
            nc.sync.dma_start(t[:], fc1wT[n])
        fc1wT_sb.append(t)
        t = cpool.tile([128, 32], F32, tag=f"fc1b{n}")
        if n == 0:
            nc.sync.dma_start(t[:], fc1b[n])
        fc1b_sb.append(t)
        ts, tb = [], []
        for c in range(2):
            a = cpool.tile([128, 1], F32, tag=f"bns{n}{c}")
            ts.append(a)
            a = cpool.tile([128, 1], F32, tag=f"bnb{n}{c}")
            tb.append(a)
        bns_sb.append(ts)
        bnb_sb.append(tb)

    def load_deferred_consts():
        nc.sync.dma_start(fc1wT_sb[1][:], fc1wT[1])
        nc.sync.dma_start(fc1b_sb[1][:], fc1b[1])
        for n in range(2):
            for c in range(2):
                nc.sync.dma_start(bns_sb[n][c][:], bns[n][c])
                nc.sync.dma_start(bnb_sb[n][c][:], bnb[n][c])
    gap16 = cpool.tile([16, BL], BF16, tag="gap16")
    ones_sb = cpool.tile([1, 64], BF16, tag="ones")
    nc.gpsimd.memset(ones_sb[:], 1.0)

    def border_memset(t):
        r = t[:].rearrange("p (h w) -> p h w", h=HP)
        nc.gpsimd.memset(r[:, 0, :], 0.0)
        nc.gpsimd.memset(r[:, HP - 1, :], 0.0)
        nc.gpsimd.memset(r[:, 1:HP - 1, 0:1], 0.0)
        nc.gpsimd.memset(r[:, 1:HP - 1, WP - 1:WP], 0.0)

    def gen_weights_a(wn, s):
        """sigmoid(fc1(gap)) -> partition-broadcast coefficient tiles."""
        aps = psmall_p.tile([128, 32], F32, tag="avec_ps")
        for j in range(32):
            nc.tensor.matmul(aps[:, j:j + 1],
                             fc1wT_sb[wn][:, 128 * j:128 * (j + 1)],
                             gap16[:, s:s + 1],
                             start=True, stop=True)
        avt = small_p.tile([128, 32], F32, tag="avtmp")
        nc.vector.tensor_add(avt[:], aps[:], fc1b_sb[wn][:])
        avec = small_p.tile([128, 32], BF16, tag="avec")
        nc.scalar.activation(avec[:], avt[:], AF.Sigmoid)
        avd = dram_p.tile([4096], BF16, tag="avd")
        nc.sync.dma_start(avd[:].rearrange("(j p) -> p j", p=128), avec[:])
        avlin = avlin_p.tile([1, 4096], BF16, tag="avlin")
        nc.sync.dma_start(avlin[:], avd[:].unsqueeze(0))
        avr = avlin[:].rearrange("o (co r) -> o co r", r=16)
        aexp = []
        for c in range(2):
            t = aexp_p.tile([128, 4 * 256], BF16, tag=f"aexp{c}")
            for half in range(2):
                aps2 = psmall_p.tile([128, 2 * 256], F32, tag="aexp_ps")
                for h in range(2):
                    for ii in range(2):
                        i = 2 * half + ii
                        m = 4 * (2 * c + h) + i
                        rhs = avr[:, :, m:m + 1].rearrange("o co r -> o (co r)")
                        nc.tensor.matmul(
                            aps2[64 * h:64 * (h + 1), 256 * ii:256 * (ii + 1)],
                            ones_sb[:], rhs, start=True, stop=True)
                nc.scalar.copy(t[:, 512 * half:512 * (half + 1)], aps2[:])
            aexp.append(t)
        return aexp

    def gen_weights_b(wn, aexp):
        wt = []
        for c in range(2):
            t = wgen_p.tile([128, 9 * 256], BF16, tag="wgen")

            def abid(i):
                return (aexp[c][:, 256 * i:256 * (i + 1)].unsqueeze(1)
                        .broadcast_to([128, 9, 256]))

            def k3(ap2d, lo):
                return ap2d[:, lo:lo + 2304].rearrange(
                    "p (k co) -> p k co", k=9)

            nc.vector.tensor_mul(k3(t[:], 0), k3(w2sb[wn][c][0][:], 0), abid(0))
            for i in range(1, 4):
                tmp = wtmp_p.tile([128, 9 * 256], BF16, tag="wtmp")
                nc.vector.tensor_mul(
                    k3(tmp[:], 0), k3(w2sb[wn][c][i][:], 0), abid(i))
                nc.vector.tensor_add(t[:], t[:], tmp[:])
            wt.append(t)
        return wt

    def gen_weights(wn, s):
        return gen_weights_b(wn, gen_weights_a(wn, s))

    def conv(wt, src_pads, sink):
        """9-offset shifted matmul conv; sink(cc, t, psum_tile) evacuates."""
        for cc in range(2):
            for t in range(NT):
                ps = psum_p.tile([128, NFREE], F32, tag="cps")
                first = True
                for c in range(2):
                    xr = src_pads[c][:].rearrange("p (h w) -> p h w", h=HP)
                    for kh in range(3):
                        for kw in range(3):
                            k = 3 * kh + kw
                            nc.tensor.matmul(
                                ps[:],
                                wt[c][:, 256 * k + 128 * cc:
                                      256 * k + 128 * cc + 128],
                                xr[:, TROWS * t + kh:TROWS * t + kh + TROWS,
                                   kw:kw + W],
                                start=first, stop=(c == 1 and k == 8))
                            first = False
                sink(cc, t, ps)

    def load_x_gap(s, pre=None):
        xpad = []
        gsum = []
        for c in range(2):
            if c == 0 and pre is not None:
                xs = pre
            else:
                xs = xstage_p.tile([128, NPIX], BF16, tag="xstage")
                nc.sync.dma_start(
                    xs[:],
                    x4b[s, 128 * c:128 * (c + 1)].rearrange("c h w -> c (h w)"))
            g = small_p.tile([128, 1], F32, tag="gsum")
            xp = xpad_p.tile([128, NPPAD], BF16, tag="xpad")
            border_memset(xp)
            xpr = xp[:].rearrange("p (h w) -> p h w", h=HP)
            nc.scalar.activation(xpr[:, 1:1 + H, 1:1 + W],
                                 xs[:].rearrange("p (h w) -> p h w", h=H),
                                 AF.Copy, accum_out=g[:])
            xpad.append(xp)
            gsum.append(g)
        gps = psmall_p.tile([16, 1], F32, tag="gap_ps")
        for c in range(2):
            nc.tensor.matmul(gps[:], rwT_sb[c][:], gsum[c][:],
                             start=(c == 0), stop=(c == 1))
        nc.scalar.activation(gap16[:, s:s + 1], gps[:], AF.Identity,
                             bias=rb_sb[:], scale=1.0)
        return xpad

    xpad = load_x_gap(0, pre=xs_pre)
    ax0 = gen_weights_a(0, 0)
    load_w2sb(0)
    w1 = gen_weights_b(0, ax0)
    load_deferred_consts()
    load_w2sb(1)
    xpad_next = None
    w1_next = None

    for s in range(BL):
        w2 = gen_weights(1, s)
        if s + 1 < BL:
            xpad_next = load_x_gap(s + 1)
            w1_next = gen_weights(0, s + 1)

        # ---- conv1 + bn1 + relu -> o1pad (bf16, padded) ----
        o1pad = []
        for c in range(2):
            op = o1pad_p.tile([128, NPPAD], BF16, tag="o1pad")
            border_memset(op)
            o1pad.append(op)

        def sink1(cc, t, ps):
            opr = o1pad[cc][:].rearrange("p (h w) -> p h w", h=HP)
            nc.scalar.activation(
                opr[:, TROWS * t + 1:TROWS * t + 1 + TROWS, 1:1 + W],
                ps[:].rearrange("p (h w) -> p h w", h=TROWS),
                AF.Relu, bias=bnb_sb[0][cc][:], scale=bns_sb[0][cc][:])

        conv(w1, xpad, sink1)

        # ---- conv2 + bn2 + residual + relu -> out ----
        def sink2(cc, t, ps):
            t2 = stage_p.tile([128, NFREE], F32, tag="t2")
            nc.scalar.activation(t2[:], ps[:], AF.Identity,
                                 bias=bnb_sb[1][cc][:], scale=bns_sb[1][cc][:])
            xres = stage_p.tile([128, NFREE], F32, tag="xres")
            xflat = x4[s, 128 * cc:128 * (cc + 1)].rearrange("c h w -> c (h w)")
            nc.sync.dma_start(xres[:], xflat[:, NFREE * t:NFREE * (t + 1)])
            nc.vector.tensor_add(t2[:], t2[:], xres[:])
            nc.vector.tensor_scalar_max(t2[:], t2[:], 0.0)
            oflat = out4[s, 128 * cc:128 * (cc + 1)].rearrange("c h w -> c (h w)")
            nc.sync.dma_start(oflat[:, NFREE * t:NFREE * (t + 1)], t2[:])

        conv(w2, o1pad, sink2)
        xpad = xpad_next
        w1 = w1_next

    ctx.close()


_NC_CACHE = {}


def get_program():
    if "nc" not in _NC_CACHE:
        _NC_CACHE["nc"] = build_program()
    return _NC_CACHE["nc"]


def prep_inputs(inputs):
    x = np.asarray(inputs["x"], np.float32)
    f32 = lambda a: np.ascontiguousarray(np.asarray(a, np.float32))
    bf = lambda a: np.ascontiguousarray(
        np.asarray(a, np.float32).astype(ml_dtypes.bfloat16))

    def pack_w2(fc2_w):
        w2_ = np.asarray(fc2_w, np.float32).reshape(256, 4, 64, 9, 4)
        w2h = w2_.transpose(4, 3, 1, 2, 0).reshape(4, 9, 256, 256)
        return bf(w2h.transpose(2, 0, 1, 3).reshape(2, 128, 4 * 9 * 256))

    def bn_fold(g, b, m, v):
        sc = np.asarray(g, np.float32) / np.sqrt(np.asarray(v, np.float32) + EPS)
        bia = np.asarray(b, np.float32) - np.asarray(m, np.float32) * sc
        return f32(sc.reshape(2, 128, 1)), f32(bia.reshape(2, 128, 1))

    base = {
        "rwT": f32((np.asarray(inputs["reduce_w"], np.float32).T / NPIX)
                   .reshape(2, 128, 16)),
        "rb": f32(np.asarray(inputs["reduce_b"]).reshape(16, 1)),
        "fc1wT1": bf(np.asarray(inputs["w1_fc1_w"]).T),
        "fc1wT2": bf(np.asarray(inputs["w2_fc1_w"]).T),
        "fc1b1": f32(np.asarray(inputs["w1_fc1_b"]).reshape(32, 128).T),
        "fc1b2": f32(np.asarray(inputs["w2_fc1_b"]).reshape(32, 128).T),
        "w2p1": pack_w2(inputs["w1_fc2_w"]),
        "w2p2": pack_w2(inputs["w2_fc2_w"]),
    }
    base["bns1"], base["bnb1"] = bn_fold(inputs["bn1_g"], inputs["bn1_b"],
                                         inputs["bn1_m"], inputs["bn1_v"])
    base["bns2"], base["bnb2"] = bn_fold(inputs["bn2_g"], inputs["bn2_b"],
                                         inputs["bn2_m"], inputs["bn2_v"])
    xb = x.astype(ml_dtypes.bfloat16)
    in_maps = []
    for i in range(NCORES):
        m = dict(base)
        m["x4"] = np.ascontiguousarray(x[i * BL:(i + 1) * BL])
        m["x4b"] = np.ascontiguousarray(xb[i * BL:(i + 1) * BL])
        in_maps.append(m)
    return in_maps


def kernel(**inputs):
    in_maps = prep_inputs(inputs)
    nc = get_program()
    res = bass_utils.run_bass_kernel_spmd(nc, in_maps,
                                          core_ids=list(range(NCORES)))
    out = np.concatenate([r["out4"] for r in res.results], axis=0)
    return out.astype(np.float32)

